# revision 1
# baseline (speedup 1.0000x reference)
"""LIF spike kernel for Trainium2 (Bass/Tile), data-parallel over batch on 8 cores.

Reparametrized recurrence: with v_t = u_t * 2^t and host-prescaled
x'_t = x_t * 2^t (exact power-of-2 scaling), the LIF step needs no tau
multiply:
  v_t = m'_{t-1} + x'_t ; s_t = v_t > 2^t ; m'_t = (v_t <= 2^t) * v_t

Engine findings baked in: DVE+GpSimd thrash each other's SBUF access
(concurrent ops ~3x slower), DVE+Act coexist cleanly, fp32 DVE ops are
element-rate-bound (~1.2us per [128,1024]). So per (b, t):
  s   = Act sign(v - 2^t) -> i8 slice of a paired [128,2048] out tile
  m'  = stt(v, 2^t, v, is_le, mult)   DVE, fresh tile
  v'  = tt(m', x'_{t+1}, add)         DVE, fresh tile
  or, for a few ACCUM slots, v' via software-DGE accum-DMA (m' += x' done
  by the DMA engines; 2x DMA cost but zero DVE cost) to balance lanes.
Host layout per core: x_core [C=128, B_loc=4, T*HW=8192] f32 (prescaled);
output i8 [C, B_loc, T*HW], spike decoded as (raw == 1).
"""

import numpy as np

import concourse.bacc as bacc
import concourse.mybir as mybir
from concourse.tile import TileContext
from concourse.bass_utils import run_bass_kernel_spmd

B, T, C, H, W = 32, 8, 128, 32, 32
HW = H * W
N_CORES = 8
B_LOC = B // N_CORES

f32 = mybir.dt.float32
i8 = mybir.dt.int8
op = mybir.AluOpType
AF = mybir.ActivationFunctionType

# spike-op engine per (b, t): 'a' = Act sign, 'v' = DVE tensor_scalar is_gt
S_ENG = [["a"] * 8 for _ in range(4)]
# timesteps t (>=1) whose x' arrives via accum-DMA onto m'_{t-1}
ACCUM = [set(), set(), set(), set()]

_nc_cache = None


def build_nc():
    nc = bacc.Bacc("TRN2", target_bir_lowering=False)
    x = nc.dram_tensor("x", [C, B_LOC, T * HW], f32, kind="ExternalInput")
    out = nc.dram_tensor("out", [C, B_LOC, T * HW], i8, kind="ExternalOutput")

    with TileContext(nc) as tc:
        with (
            tc.tile_pool(name="xq", bufs=5) as xq,
            tc.tile_pool(name="vp", bufs=5) as vp,
            tc.tile_pool(name="mp", bufs=3) as mp,
            tc.tile_pool(name="pp", bufs=1, space="PSUM") as pp,
            tc.tile_pool(name="sp_", bufs=3) as spool,
            tc.tile_pool(name="cst", bufs=1) as cst,
        ):
            # Act sign needs bias as a per-partition AP: -2^t for each t
            bias = []
            for t in range(T):
                bt = cst.tile([C, 1], f32, name=f"bias{t}")
                nc.vector.memset(bt[:], -float(2**t))
                bias.append(bt)

            # preload the activation table before data arrives
            warm = cst.tile([C, 1], i8, name="warm")
            nc.scalar.activation(warm[:], bias[0][:], AF.Sign, bias=bias[0][:])

            # t=0 chunks land in the v pool directly (v_0 = x'_0),
            # interleaved with each chain's x'_1 fetch so round-0 adds
            # aren't queued behind all four v_0 transfers
            v_cur = [None] * B_LOC
            xt_tiles = [[None] * T for _ in range(B_LOC)]

            def issue_one(b, t):
                xt = xq.tile([C, HW], f32, tag=f"x{b}", name=f"x_{b}_{t}")
                nc.sync.dma_start(out=xt[:], in_=x[:, b, t * HW : (t + 1) * HW])
                xt_tiles[b][t] = xt

            for b in range(B_LOC):
                vt = vp.tile([C, HW], f32, tag=f"v{b}", name=f"v0_{b}")
                nc.sync.dma_start(out=vt[:], in_=x[:, b, 0:HW])
                v_cur[b] = vt
                if 1 not in ACCUM[b]:
                    issue_one(b, 1)

            def issue_in(t):
                # just-in-time x'_t fetches (skipping accum-covered slots) so
                # out-DMAs interleave with in-DMAs on the SP queue
                if 1 <= t < T:
                    for b in range(B_LOC):
                        if t in ACCUM[b]:
                            continue
                        issue_one(b, t)

            issue_in(2)

            s_tiles = [None] * B_LOC
            for t in range(T):
                issue_in(t + 3)
                thr = float(2**t)
                for b in range(B_LOC):
                    v = v_cur[b]
                    # spike output into a paired [C, 2HW] i8 tile (t even:
                    # allocate; t odd: fill second half then DMA out)
                    if t % 2 == 0:
                        s_tiles[b] = spool.tile([C, 2 * HW], i8, tag=f"s{b}", name=f"s{b}_{t}")
                    st = s_tiles[b][:, (t % 2) * HW : (t % 2 + 1) * HW]
                    if S_ENG[b][t] == "a":
                        nc.scalar.activation(
                            st, v[:], AF.Sign, bias=bias[t][:], scale=1.0
                        )
                    else:
                        nc.vector.tensor_scalar(st, v[:], thr, None, op.is_gt)
                    if t == T - 1:
                        nc.sync.dma_start(
                            out=out[:, b, (t - 1) * HW : t * HW],
                            in_=s_tiles[b][:, 0:HW],
                        )
                        nc.sync.dma_start(
                            out=out[:, b, t * HW : (t + 1) * HW],
                            in_=s_tiles[b][:, HW : 2 * HW],
                        )
                    elif t % 2 == 1:
                        nc.sync.dma_start(
                            out=out[:, b, (t - 1) * HW : (t + 1) * HW],
                            in_=s_tiles[b][:],
                        )
                    if t == T - 1:
                        continue
                    # m' = (v <= 2^t) * v ; v' = m' + x'_{t+1}
                    # m' lives in PSUM (separate memory -> less SBUF port
                    # contention with DMA) except for accum slots, which the
                    # gpsimd software DGE can only write in SBUF
                    if (t + 1) in ACCUM[b]:
                        mt = mp.tile([C, HW], f32, tag=f"m{b}", name=f"m_{b}_{t}")
                    else:
                        mt = pp.tile([C, HW], f32, tag=f"pm{b}", name=f"pm_{b}_{t}")
                    nc.vector.scalar_tensor_tensor(
                        mt[:], v[:], thr, v[:], op.is_le, op.mult
                    )
                    if (t + 1) in ACCUM[b]:
                        nc.gpsimd.dma_start(
                            out=mt[:],
                            in_=x[:, b, (t + 1) * HW : (t + 2) * HW],
                            accum_op=op.add,
                        )
                        v_cur[b] = mt
                    else:
                        vn = vp.tile([C, HW], f32, tag=f"v{b}")
                        nc.vector.tensor_tensor(
                            vn[:], mt[:], xt_tiles[b][t + 1][:], op.add
                        )
                        v_cur[b] = vn
    nc.compile()
    return nc


def make_in_maps(x: np.ndarray) -> list[dict]:
    xs = np.ascontiguousarray(x).reshape(B, T, C, HW)
    # prescale x'_t = x_t * 2^t (exact in f32)
    scale = (2.0 ** np.arange(T, dtype=np.float32)).astype(np.float32)
    xs = (xs * scale[None, :, None, None]).astype(np.float32)
    return [
        {
            "x": np.ascontiguousarray(
                xs[i * B_LOC : (i + 1) * B_LOC].transpose(2, 0, 1, 3)
            ).reshape(C, B_LOC, T * HW)
        }
        for i in range(N_CORES)
    ]


def kernel(x: np.ndarray) -> np.ndarray:
    global _nc_cache
    if _nc_cache is None:
        _nc_cache = build_nc()
    res = run_bass_kernel_spmd(_nc_cache, make_in_maps(x), list(range(N_CORES)))
    # out[c, b_loc, t*HW+hw] -> [b, t, c, hw]; spike iff raw == 1
    parts = [
        (res.results[i]["out"].reshape(C, B_LOC, T, HW) == 1).transpose(1, 2, 0, 3)
        for i in range(N_CORES)
    ]
    full = np.concatenate(parts, axis=0)
    return full.reshape(B, T, C, H, W).astype(np.float32)



# revision 3
# speedup vs baseline: 1.3030x; 1.3030x over previous
"""LIF spike kernel for Trainium2 (Bass/Tile), data-parallel over batch on 8 cores.

Reparametrized recurrence: v_t = u_t * 2^t with host-prescaled
x'_t = x_t * 2^t (exact power-of-2 scaling), so the step is
  v_{t+1} = (v_t <= 2^t) * v_t + x'_{t+1}
computed by ONE fused custom-DVE op (LIF_STEP_ANT). Spikes are emitted
as pair-packed ternary bytes by a second fused op (LIF_PACK2_ANT):
  byte_p = d(v_{2p}, 2^{2p}) + 3 * d(v_{2p+1}, 2^{2p+1}),
  d(v, th) = (v > th) - (v < th) in {-1, 0, 1}
so the output is 1 byte per 2 timesteps (2 MiB/core instead of 4) and
the Act engine is not needed at all. Host decodes d == 1 as spike.

Per-core layout: x [C=128, T=8, M=4096] f32 (M = B_loc*HW, prescaled),
out [C, 4, M] i8. Free dim processed in 2 chunks of 2048 so compute
pipelines behind the DMA in-stream.
"""

import numpy as np

import concourse.bacc as bacc
import concourse.mybir as mybir
from concourse.tile import TileContext
from concourse.bass_utils import run_bass_kernel_spmd

import concourse.dve_ops as dve_ops_mod
from concourse.dve_ops import DveOp, OPS, CUSTOM_DVE_SPECS
from concourse.dve_spec import Spec, Src0, Src1, C0, C1, C2, lower, _has_src1
from concourse.dve_uop import DveOpSpec

B, T, C, H, W = 32, 8, 128, 32, 32
HW = H * W
N_CORES = 8
B_LOC = B // N_CORES
M = B_LOC * HW  # free dim per (c, t): 4096
CH = 2048  # chunk of the free dim
NCH = M // CH

f32 = mybir.dt.float32
i8 = mybir.dt.int8


def _register(name, spec, subdim=False):
    existing = {op.name: op for op in OPS}
    if name in existing:
        return existing[name]
    row = dve_ops_mod._CUSTOM_DVE_ROW_BASE + len(OPS)
    assert row < 0x20, "no free custom-DVE rows"
    dve_ops_mod._SUB_OPCODE_FOR_NAME[name] = row
    shas = {}
    for ver in ("v3", "v4"):
        uops = lower(spec, ver=ver)
        shas[ver] = DveOpSpec(
            name=name, opcode=row, uops=uops, rd1_en=_has_src1(spec)
        ).sha(ver)
    op = DveOp(name, spec, subdim, uops_sha=shas)
    OPS.append(op)
    CUSTOM_DVE_SPECS[name] = spec
    return op


STEP = _register(
    "LIF_STEP_ANT",
    Spec(
        body=(Src0 * (Src0 <= C0)) + Src1,
        reference=lambda in0, in1, s0, s1, imm2: in0 * (in0 <= s0) + in1,
    ),
)

_d0 = (Src0 > C0) - (Src0 < C0)
_d1 = (Src1 > C1) - (Src1 < C1)
PACK2 = _register(
    "LIF_PACK2_ANT",
    Spec(
        body=_d0 + (_d1 * C2),
        reference=lambda in0, in1, s0, s1, imm2: (
            (in0 > s0).astype(np.float32)
            - (in0 < s0)
            + imm2 * ((in1 > s1).astype(np.float32) - (in1 < s1))
        ),
    ),
)

_nc_cache = None


def build_nc():
    nc = bacc.Bacc("TRN2", target_bir_lowering=False)
    x = nc.dram_tensor("x", [C, T, M], f32, kind="ExternalInput")
    out = nc.dram_tensor("out", [C, T // 2, M], i8, kind="ExternalOutput")

    with TileContext(nc) as tc:
        with (
            tc.tile_pool(name="xq", bufs=4) as xq,
            tc.tile_pool(name="vp", bufs=4) as vp,
            tc.tile_pool(name="op_", bufs=4) as opool,
        ):
            xt = {}

            def fetch(t):
                for ch in range(NCH):
                    xx = xq.tile([C, CH], f32, tag=f"x{ch}", name=f"x_{t}_{ch}")
                    nc.sync.dma_start(
                        out=xx[:], in_=x[:, t, ch * CH : (ch + 1) * CH]
                    )
                    xt[(t, ch)] = xx

            # v_0 = x'_0 lands directly in the v pool
            v_cur = []
            v_prev = [None] * NCH
            for ch in range(NCH):
                vt = vp.tile([C, CH], f32, tag=f"v{ch}", name=f"v0_{ch}")
                nc.sync.dma_start(out=vt[:], in_=x[:, 0, ch * CH : (ch + 1) * CH])
                v_cur.append(vt)
            fetch(1)
            fetch(2)

            for t in range(T):
                if t + 3 < T:
                    fetch(t + 3)
                v_now = list(v_cur)  # v_t
                # advance the serial chain first (critical path) ...
                if t < T - 1:
                    for ch in range(NCH):
                        vn = vp.tile([C, CH], f32, tag=f"v{ch}", name=f"v{t + 1}_{ch}")
                        nc.vector._custom_dve(
                            STEP,
                            out=vn[:],
                            in0=v_now[ch][:],
                            in1=xt[(t + 1, ch)][:],
                            s0=float(2**t),
                        )
                        v_cur[ch] = vn
                # ... then emit the pair-packed spikes for (t-1, t) at odd t
                if t % 2 == 1:
                    for ch in range(NCH):
                        ob = opool.tile([C, CH], i8, tag=f"o{ch}", name=f"o{t // 2}_{ch}")
                        nc.vector._custom_dve(
                            PACK2,
                            out=ob[:],
                            in0=v_prev[ch][:],
                            in1=v_now[ch][:],
                            s0=float(2 ** (t - 1)),
                            s1=float(2**t),
                            imm2=3.0,
                        )
                        nc.sync.dma_start(
                            out=out[:, t // 2, ch * CH : (ch + 1) * CH],
                            in_=ob[:],
                        )
                v_prev = v_now
    nc.compile()
    return nc


def make_in_maps(x: np.ndarray) -> list[dict]:
    xs = np.ascontiguousarray(x).reshape(B, T, C, HW)
    scale = (2.0 ** np.arange(T, dtype=np.float32)).astype(np.float32)
    xs = (xs * scale[None, :, None, None]).astype(np.float32)
    return [
        {
            "x": np.ascontiguousarray(
                xs[i * B_LOC : (i + 1) * B_LOC].transpose(2, 1, 0, 3)
            ).reshape(C, T, M)
        }
        for i in range(N_CORES)
    ]


def kernel(x: np.ndarray) -> np.ndarray:
    global _nc_cache
    if _nc_cache is None:
        _nc_cache = build_nc()
    res = run_bass_kernel_spmd(_nc_cache, make_in_maps(x), list(range(N_CORES)))
    parts = []
    for i in range(N_CORES):
        raw = res.results[i]["out"].reshape(C, T // 2, B_LOC, HW).astype(np.int16)
        r = raw + 4  # (d_e + 1) + 3 * (d_o + 1) in [0, 8]
        s = np.empty((T, C, B_LOC, HW), dtype=bool)
        for p in range(T // 2):
            s[2 * p] = r[:, p] % 3 == 2
            s[2 * p + 1] = r[:, p] // 3 == 2
        parts.append(s.transpose(2, 0, 1, 3))  # [B_LOC, T, C, HW]
    full = np.concatenate(parts, axis=0)
    return full.reshape(B, T, C, H, W).astype(np.float32)


# revision 6
# speedup vs baseline: 1.3392x; 1.0278x over previous
"""LIF spike kernel for Trainium2 (Bass/Tile), data-parallel over batch on 8 cores.

Reparametrized recurrence: v_t = u_t * 2^t with host-prescaled
x'_t = x_t * 2^t (exact power-of-2 scaling), so the step is
  v_{t+1} = (v_t <= 2^t) * v_t + x'_{t+1}
computed by ONE fused custom-DVE op (LIF_STEP_ANT). Spikes are emitted
as pair-packed ternary bytes by a second fused op (LIF_PACK2_ANT):
  byte_p = d(v_{2p}, 2^{2p}) + 3 * d(v_{2p+1}, 2^{2p+1}),
  d(v, th) = (v > th) - (v < th) in {-1, 0, 1}
so the output is 1 byte per 2 timesteps (2 MiB/core instead of 4) and
only the DVE computes. Host decodes d == 1 as spike.

Per-core layout: x [C=128, T=8, M=4096] f32 (M = B_loc*HW, prescaled),
out [C, 4, M] i8. Early timesteps are sub-split (t0 into 8, t1 into 4,
t2 into 2 slices) so the first STEP starts after ~0.5 MiB of input
instead of 2 MiB; later timesteps run as single [128, 4096] ops.
"""

import numpy as np

import concourse.bacc as bacc
import concourse.mybir as mybir
from concourse.tile import TileContext
from concourse.bass_utils import run_bass_kernel_spmd

import concourse.dve_ops as dve_ops_mod
from concourse.dve_ops import DveOp, OPS, CUSTOM_DVE_SPECS
from concourse.dve_spec import Spec, Src0, Src1, C0, C1, C2, lower, _has_src1
from concourse.dve_uop import DveOpSpec

B, T, C, H, W = 32, 8, 128, 32, 32
HW = H * W
N_CORES = 8
B_LOC = B // N_CORES
M = B_LOC * HW  # free dim per (c, t): 4096

# sub-splits per timestep index (for x_t fetches and the ops reading x_t)
SPLITS = {0: 8, 1: 8, 2: 2}

f32 = mybir.dt.float32
i8 = mybir.dt.int8


def _register(name, spec, subdim=False):
    existing = {op.name: op for op in OPS}
    if name in existing:
        return existing[name]
    row = dve_ops_mod._CUSTOM_DVE_ROW_BASE + len(OPS)
    assert row < 0x20, "no free custom-DVE rows"
    dve_ops_mod._SUB_OPCODE_FOR_NAME[name] = row
    shas = {}
    for ver in ("v3", "v4"):
        uops = lower(spec, ver=ver)
        shas[ver] = DveOpSpec(
            name=name, opcode=row, uops=uops, rd1_en=_has_src1(spec)
        ).sha(ver)
    op = DveOp(name, spec, subdim, uops_sha=shas)
    OPS.append(op)
    CUSTOM_DVE_SPECS[name] = spec
    return op


STEP = _register(
    "LIF_STEP_ANT",
    Spec(
        body=(Src0 * (Src0 <= C0)) + Src1,
        reference=lambda in0, in1, s0, s1, imm2: in0 * (in0 <= s0) + in1,
    ),
)

_d0 = (Src0 > C0) - (Src0 < C0)
_d1 = (Src1 > C1) - (Src1 < C1)
PACK2 = _register(
    "LIF_PACK2_ANT",
    Spec(
        body=_d0 + (_d1 * C2),
        reference=lambda in0, in1, s0, s1, imm2: (
            (in0 > s0).astype(np.float32)
            - (in0 < s0)
            + imm2 * ((in1 > s1).astype(np.float32) - (in1 < s1))
        ),
    ),
)

_nc_cache = None


def build_nc():
    nc = bacc.Bacc("TRN2", target_bir_lowering=False)
    x = nc.dram_tensor("x", [C, T, M], f32, kind="ExternalInput")
    out = nc.dram_tensor("out", [C, T // 2, M], i8, kind="ExternalOutput")

    with TileContext(nc) as tc:
        with (
            tc.tile_pool(name="xq", bufs=7) as xq,
            tc.tile_pool(name="vp", bufs=4) as vp,
            tc.tile_pool(name="op_", bufs=4) as opool,
        ):
            def subs(t):
                n = SPLITS.get(t, 1)
                w = M // n
                return [(k * w, (k + 1) * w) for k in range(n)]

            # v_0 = x'_0 lands directly in the v pool; interleave its
            # sub-fetches with x_1's so the first STEP slice is ready after
            # ~0.5 MiB of input
            v_cur = vp.tile([C, M], f32, tag="v", name="v0")
            x1 = xq.tile([C, M], f32, tag="x", name="x_1")
            xt = {1: x1}
            for a0, a1 in subs(0):
                nc.sync.dma_start(out=v_cur[:, a0:a1], in_=x[:, 0, a0:a1])
                nc.sync.dma_start(out=x1[:, a0:a1], in_=x[:, 1, a0:a1])
            for t in range(2, T):
                xx = xq.tile([C, M], f32, tag="x", name=f"x_{t}")
                for a0, a1 in subs(t):
                    nc.sync.dma_start(out=xx[:, a0:a1], in_=x[:, t, a0:a1])
                xt[t] = xx

            v_prev = None
            for t in range(T):
                v_now = v_cur
                if t < T - 1:
                    vn = vp.tile([C, M], f32, tag="v", name=f"v{t + 1}")
                    for a0, a1 in subs(t + 1):
                        nc.vector._custom_dve(
                            STEP,
                            out=vn[:, a0:a1],
                            in0=v_now[:, a0:a1],
                            in1=xt[t + 1][:, a0:a1],
                            s0=float(2**t),
                        )
                    v_cur = vn
                if t % 2 == 1:
                    ob = opool.tile([C, M], i8, tag="o", name=f"o{t // 2}")
                    for a0, a1 in subs(t):
                        nc.vector._custom_dve(
                            PACK2,
                            out=ob[:, a0:a1],
                            in0=v_prev[:, a0:a1],
                            in1=v_now[:, a0:a1],
                            s0=float(2 ** (t - 1)),
                            s1=float(2**t),
                            imm2=3.0,
                        )
                        nc.sync.dma_start(
                            out=out[:, t // 2, a0:a1], in_=ob[:, a0:a1]
                        )
                v_prev = v_now
    nc.compile()
    return nc


def make_in_maps(x: np.ndarray) -> list[dict]:
    xs = np.ascontiguousarray(x).reshape(B, T, C, HW)
    scale = (2.0 ** np.arange(T, dtype=np.float32)).astype(np.float32)
    xs = (xs * scale[None, :, None, None]).astype(np.float32)
    return [
        {
            "x": np.ascontiguousarray(
                xs[i * B_LOC : (i + 1) * B_LOC].transpose(2, 1, 0, 3)
            ).reshape(C, T, M)
        }
        for i in range(N_CORES)
    ]


def kernel(x: np.ndarray) -> np.ndarray:
    global _nc_cache
    if _nc_cache is None:
        _nc_cache = build_nc()
    res = run_bass_kernel_spmd(_nc_cache, make_in_maps(x), list(range(N_CORES)))
    parts = []
    for i in range(N_CORES):
        raw = res.results[i]["out"].reshape(C, T // 2, B_LOC, HW).astype(np.int16)
        r = raw + 4  # (d_e + 1) + 3 * (d_o + 1) in [0, 8]
        s = np.empty((T, C, B_LOC, HW), dtype=bool)
        for p in range(T // 2):
            s[2 * p] = r[:, p] % 3 == 2
            s[2 * p + 1] = r[:, p] // 3 == 2
        parts.append(s.transpose(2, 0, 1, 3))  # [B_LOC, T, C, HW]
    full = np.concatenate(parts, axis=0)
    return full.reshape(B, T, C, H, W).astype(np.float32)
